# revision 33
# baseline (speedup 1.0000x reference)
"""Trainium2 Bass kernel for nn_CA_event (CA_event.forward batched ODE RHS).

reference:
    x   = state[:, 0:100]
    e_x = state[:, 100:200]
    W_a = state[:, 300:400]          (W_c = state[:, 200:300] unused)
    u   = W_a * (x + e_x - target)
    s   = x^2 / (1 + x^2)
    dx  = -x + s @ A.T + u * s
    out = concat([dx, -dx, 0, 0], axis=-1)      # [B, 400]

Strategy: pure data parallel over 8 NeuronCores (batch 131072 -> 16384
rows/core); A and target replicated.

Layout: the host stages each core's shard FEATURE-MAJOR (transposed) and
f16: state_dev = [300, 16384] = [xT | eT | wT].  This makes the kernel
DMA-roofline-shaped on device:
  * loads/stores are fully contiguous 4KB-per-partition descriptors;
  * the contraction dim of s@A.T lands on partitions, so the matmul runs
    with A.T as a resident stationary operand -- no per-group PE
    transposes, no PSUM->SBUF staging copies;
  * target / sum_k A[j,k] become per-partition scalars, folded into a
    fused scalar_tensor_tensor op and the output writes' bias for free.

Math (rm1 := 1/(1+x^2) - 1 = -s, computed by one fused custom-DVE op:
bitwise-NOT Chebyshev seed + one Newton pass, ~1e-3 rel):
    PSUM_he = I@xT + I@eT            (TensorE identity-matmul accumulation)
    u  = (PSUM_he - tgt) * w         (DVE stt, tgt per-partition scalar)
    t  = rm1 * u  = -u*s             (Pool tensor_mul)
    PSUM = I@xT + I@tT + A.T-matmul(rm1T)        (TensorE, 3 f16 matmuls)
         = xT - (u*s)T - (s@A.T).T = -dxT
  (sum_k A[j,k]*(r[c,k]-1) = -(s@A.T).T[j,c] exactly)
    -dxT -> out[100:200,:]   (ACT copy);   dxT = -PSUM -> out[0:100,:]
  Engine balance per pass: DMA 45.5us (bound) > PE ~38 > ACT ~37 >
  DVE ~33 > Pool ~33 (model).

The device emits only the data-dependent half of the output (dxT | -dxT,
f16, ~5e-4 rel << the 2e-2 gate); the host upcasts/untransposes and
supplies the structurally-zero half (derivatives of W_c / W_a are
identically 0 for any input).
"""

import os
import sys

try:
    import concourse  # noqa: F401  (resolves via the environment's default path)
except ImportError:  # fall back for bare environments
    sys.path.insert(0, "/opt/trn_rl_repo")

import numpy as np

import concourse.bass as bass
import concourse.bacc as bacc
import concourse.mybir as mybir
from concourse import tile
from concourse import masks

DIM = 100
PACK = 3 * DIM                           # xT | eT | wT rows on device
BATCH = 131072
NCORES = 8
ROWS_PER_CORE = BATCH // NCORES          # 16384

F32 = mybir.dt.float32
F16 = mybir.dt.float16

_RUNNERS = {}  # key -> runner dict
_CA_OPS = None


def _register_ca_ops():
    """Register a fused custom-DVE op computing rm1 = 1/(1+x^2) - 1 from x.

    CA_RM1_NR1: in0=x -> r - 1 = -s   (Chebyshev bitwise-NOT seed + 1 NR
    pass, ~1e-3 rel).  Same math/constants as
    dve_ops.RECIPROCAL_APPROX_FAST with the (1 + x^2) denominator
    computation and the final -1 folded in.  Registered at runtime
    (appended to dve_ops.OPS) so no repo files change.
    """
    global _CA_OPS
    if _CA_OPS is not None:
        return _CA_OPS
    from concourse import dve_ops
    from concourse.dve_spec import Spec, Src0, C0, C1, One, Bin, AluOp, sq
    from concourse.dve_uop import DveOpSpec

    d = sq(Src0) + One
    nd = Bin(AluOp.BITWISE_NOT, d, d)
    y0 = nd * C0
    body = y0 * (C1 - d * y0) - One

    def ref(in0, in1, s0, s1, imm2):
        dd = (1.0 + in0.astype(np.float32) * in0).astype(np.float32)
        ndd = (~dd.view(np.int32)).view(np.float32)
        yy0 = (ndd * np.float32(s0)).astype(np.float32)
        return (yy0 * (np.float32(s1) - dd * yy0) - 1.0).astype(np.float32)

    ops = []
    for name, spec in [("CA_RM1_NR1", Spec(body=body, reference=ref))]:
        if name not in dve_ops._SUB_OPCODE_FOR_NAME:
            row = max(dve_ops._SUB_OPCODE_FOR_NAME.values()) + 1
            assert row < 0x20
            dve_ops._SUB_OPCODE_FOR_NAME[name] = row
        shas = {}
        for ver in ("v3", "v4"):
            s = DveOpSpec(
                name=name,
                opcode=dve_ops.get_dve_sub_opcode(name),
                uops=dve_ops.lower(spec, ver=ver),
                rd1_en=dve_ops.has_src1(spec),
            )
            shas[ver] = s.sha(ver)
        op = dve_ops.DveOp(name, spec, subdim=False, uops_sha=shas)
        if not any(o.name == name for o in dve_ops.OPS):
            dve_ops.OPS.append(op)
            dve_ops.CUSTOM_DVE_SPECS[name] = spec
        ops.append(op)
    _CA_OPS = tuple(ops)
    return _CA_OPS


def _build(repeat=1, ablate=(), loop_k=1, f_tile=2048, he_mode="pe",
           u_eng="dve", t_eng="pool", store_ring="split", load_ring="sp",
           body_unroll=8, merged=True):
    """Build the per-core Bacc module.

    he_mode: engine computing he = x + e: 'pool' | 'dve' | 'pe' (PSUM
             identity-matmul accumulation, freeing the elementwise engines)
    u_eng:   engine for u = (he - tgt) * w: 'dve' | 'pool'
             (must be 'dve' when he_mode='pe' -- GpSimd cannot read PSUM)
    t_eng:   engine for t = (r - 1) * u: 'dve' | 'pool'
    body_unroll: passes per For_i iteration when loop_k > 1 -- For_i does an
             all-engine barrier + semaphore reset each iteration (pipeline
             drain); unrolling amortizes it
    ablate: stages to skip for timing experiments only (output wrong):
            'dve', 'pe', 'act', 'load', 'store'
    """
    ablate = set(ablate)
    F = f_tile
    NTILES = ROWS_PER_CORE // F
    CH = 512                              # matmul chunk (one f32 PSUM bank)
    NCH = F // CH
    nc = bacc.Bacc("TRN2", target_bir_lowering=False, debug=False)

    if merged:
        # x/e/w chunk-interleaved on 100 partitions: one 12KB-per-partition
        # load and one 8KB-per-partition store per tile (fewer, bigger DMAs)
        state = nc.declare_dram_parameter("state", [DIM, 3 * ROWS_PER_CORE], F16, isOutput=False)
        out = nc.declare_dram_parameter("out", [DIM, 2 * ROWS_PER_CORE], F16, isOutput=True)
    else:
        state = nc.declare_dram_parameter("state", [PACK, ROWS_PER_CORE], F16, isOutput=False)
        out = nc.declare_dram_parameter("out", [2 * DIM, ROWS_PER_CORE], F16, isOutput=True)
    A = nc.declare_dram_parameter("A", [DIM, DIM], F32, isOutput=False)
    target = nc.declare_dram_parameter("target", [DIM], F32, isOutput=False)

    st_ap = state.ap()
    out_ap = out.ap()

    (op_r,) = _register_ca_ops()

    rings = {"sp": nc.sync, "pool": nc.gpsimd, "act": nc.scalar, "dve": nc.vector}
    ld = rings[load_ring]
    sr = rings.get(store_ring)

    with tile.TileContext(nc) as tc:
        with (
            tc.tile_pool(name="consts", bufs=1) as consts,
            tc.tile_pool(name="inp", bufs=3) as inp,
            tc.tile_pool(name="work", bufs=3) as work,
            tc.tile_pool(name="outp", bufs=3) as outp,
            tc.tile_pool(name="psum_mm", bufs=4, space="PSUM") as psum_mm,
        ):
            # ---- one-time constants -------------------------------------
            idf = consts.tile([DIM, DIM], F32)
            masks.make_identity(nc, idf[:])
            id16 = consts.tile([DIM, DIM], F16)
            nc.scalar.copy(id16[:], idf[:])

            a_sb = consts.tile([DIM, DIM], F32)
            nc.sync.dma_start(out=a_sb[:], in_=A.ap())

            # A^T (f16 stationary for the per-chunk matmuls)
            a_ps = psum_mm.tile([DIM, DIM], F32, tag="mm")
            nc.tensor.transpose(a_ps[:], a_sb[:], idf[:])
            at16 = consts.tile([DIM, DIM], F16)
            nc.scalar.copy(at16[:], a_ps[:])

            # target as a per-partition scalar [100, 1]
            tgt = consts.tile([DIM, 1], F32)
            nc.sync.dma_start(out=tgt[:], in_=target.ap()[:, None])

            # ---- main loop ----------------------------------------------
            def emit_pass():
                for i in range(NTILES):
                    sl = slice(i * F, (i + 1) * F)
                    if merged:
                        xew = inp.tile([DIM, 3 * F], F16, tag="xew")
                        if "load" not in ablate:
                            ld.dma_start(out=xew[:],
                                         in_=st_ap[:, i * 3 * F:(i + 1) * 3 * F])
                        xt = xew[:, 0:F]
                        et = xew[:, F:2 * F]
                        wt = xew[:, 2 * F:3 * F]
                    else:
                        xt = inp.tile([DIM, F], F16, tag="x")
                        et = inp.tile([DIM, F], F16, tag="e")
                        wt = inp.tile([DIM, F], F16, tag="w")
                        if "load" not in ablate:
                            ld.dma_start(out=xt[:], in_=st_ap[0:DIM, sl])
                            ld.dma_start(out=et[:], in_=st_ap[DIM:2 * DIM, sl])
                            ld.dma_start(out=wt[:], in_=st_ap[2 * DIM:3 * DIM, sl])

                    if merged:
                        if "act" not in ablate:
                            dn = outp.tile([DIM, 2 * F], F16, tag="dn")
                            dx_sb = dn[:, 0:F]
                            ndx_sb = dn[:, F:2 * F]
                        else:
                            dn = xew[:, 0:2 * F]
                            dx_sb, ndx_sb = xt, et
                    elif "act" not in ablate:
                        dx_sb = outp.tile([DIM, F], F16, tag="dx")
                        ndx_sb = outp.tile([DIM, F], F16, tag="ndx")
                    else:
                        dx_sb, ndx_sb = xt, et   # timing-only: store inputs

                    u = work.tile([DIM, F], F16, tag="u")
                    t = work.tile([DIM, F], F16, tag="t")
                    rm1 = work.tile([DIM, F], F16, tag="rm1")
                    # scalar_tensor_tensor (TensorScalarPtr) is DVE-only on
                    # HW; GpSimd additionally cannot read PSUM.  Pool gets
                    # only plain TensorTensor ops on SBUF.
                    assert u_eng == "dve"
                    t_e = nc.gpsimd if t_eng == "pool" else nc.vector
                    if "dve" not in ablate:
                        # rm1 = 1/(1+x^2) - 1 = -s
                        nc.vector._custom_dve(
                            op_r, out=rm1[:], in0=xt[:],
                            s0=float(np.float32(-0.23549792)),
                            s1=float(np.float32(2.0017324)),
                        )
                        if he_mode == "pe":
                            # he = x + e lives in PSUM via identity matmuls
                            for j in range(NCH):
                                js = slice(j * CH, (j + 1) * CH)
                                ph = psum_mm.tile([DIM, CH], F32, tag="he",
                                                  bufs=2)
                                nc.tensor.matmul(ph[:], id16[:], xt[:, js],
                                                 start=True, stop=False,
                                                 skip_group_check=True)
                                nc.tensor.matmul(ph[:], id16[:], et[:, js],
                                                 start=False, stop=True,
                                                 skip_group_check=True)
                                # u = (he - tgt) * w
                                nc.vector.scalar_tensor_tensor(
                                    u[:, js], ph[:], tgt[:], wt[:, js],
                                    op0=mybir.AluOpType.subtract,
                                    op1=mybir.AluOpType.mult,
                                )
                                # t = rm1 * u = -u*s
                                t_e.tensor_mul(t[:, js], rm1[:, js], u[:, js])
                        else:
                            he = work.tile([DIM, F], F16, tag="he")
                            he_e = nc.gpsimd if he_mode == "pool" else nc.vector
                            he_e.tensor_add(he[:], xt[:], et[:])
                            # u = (he - tgt) * w
                            nc.vector.scalar_tensor_tensor(
                                u[:], he[:], tgt[:], wt[:],
                                op0=mybir.AluOpType.subtract,
                                op1=mybir.AluOpType.mult,
                            )
                            # t = rm1 * u = -u*s
                            t_e.tensor_mul(t[:], rm1[:], u[:])

                    for j in range(NCH):
                        js = slice(j * CH, (j + 1) * CH)
                        mm = psum_mm.tile([DIM, CH], F32, tag="mm")
                        if "pe" not in ablate:
                            nc.tensor.matmul(mm[:], id16[:], xt[:, js],
                                             start=True, stop=False,
                                             skip_group_check=True)
                            nc.tensor.matmul(mm[:], id16[:], t[:, js],
                                             start=False, stop=False,
                                             skip_group_check=True)
                            nc.tensor.matmul(mm[:], at16[:], rm1[:, js],
                                             start=False, stop=True,
                                             skip_group_check=True)
                        if "act" not in ablate:
                            # psum = x - u*s + (A @ rm1T) = -dxT exactly
                            # (sum_k A[j,k](r-1) = -(s@A.T).T)
                            nc.scalar.copy(ndx_sb[:, js], mm[:])
                            nc.scalar.mul(dx_sb[:, js], mm[:], -1.0)

                    if "store" not in ablate:
                        if merged:
                            st_e = nc.scalar if store_ring in ("split", "act") else (
                                nc.gpsimd if store_ring == "pool" else nc.sync)
                            st_e.dma_start(out=out_ap[:, i * 2 * F:(i + 1) * 2 * F],
                                           in_=dn)
                        elif store_ring == "split":
                            nc.scalar.dma_start(out=out_ap[0:DIM, sl], in_=dx_sb[:])
                            nc.sync.dma_start(out=out_ap[DIM:2 * DIM, sl], in_=ndx_sb[:])
                        else:
                            sr.dma_start(out=out_ap[0:DIM, sl], in_=dx_sb[:])
                            sr.dma_start(out=out_ap[DIM:2 * DIM, sl], in_=ndx_sb[:])

            if loop_k > 1:
                stag = bool(int(os.environ.get("CA_STAG", "0")))
                bu = body_unroll
                n_iter = loop_k // bu
                rem = loop_k - n_iter * bu
                if n_iter > 0:
                    with tc.For_i(0, n_iter, 1, staggered_reset=stag):
                        for _ in range(bu):
                            emit_pass()
                for _ in range(rem):
                    emit_pass()
            else:
                for _ in range(repeat):
                    emit_pass()

    nc.compile()
    return nc


def _make_runner(nc):
    """Cached jitted shard_map executor for a prebuilt Bacc module.

    Mirrors bass2jax.run_bass_via_pjrt, but keeps the jitted callable (and
    device-resident inputs) reusable across calls so repeated invocations
    don't re-trace/re-compile.
    """
    import jax
    from jax.experimental.shard_map import shard_map
    from jax.sharding import Mesh, PartitionSpec
    from concourse import bass2jax

    bass2jax.install_neuronx_cc_hook()

    partition_name = nc.partition_id_tensor.name if nc.partition_id_tensor else None
    in_names, out_names, out_avals, zero_shapes = [], [], [], []
    for alloc in nc.m.functions[0].allocations:
        if not isinstance(alloc, mybir.MemoryLocationSet):
            continue
        name = alloc.memorylocations[0].name
        if alloc.kind == "ExternalInput":
            if name != partition_name:
                in_names.append(name)
        elif alloc.kind == "ExternalOutput":
            out_names.append(name)
            shape = tuple(alloc.tensor_shape)
            dtype = mybir.dt.np(alloc.dtype)
            out_avals.append(jax.core.ShapedArray(shape, dtype))
            zero_shapes.append((shape, dtype))
    n_params = len(in_names)
    n_outs = len(out_names)
    bind_in_names = list(in_names) + list(out_names)
    if partition_name is not None:
        bind_in_names.append(partition_name)

    def _body(*args):
        operands = list(args)
        if partition_name is not None:
            operands.append(bass2jax.partition_id_tensor())
        outs = bass2jax._bass_exec_p.bind(
            *operands,
            out_avals=tuple(out_avals),
            in_names=tuple(bind_in_names),
            out_names=tuple(out_names),
            lowering_input_output_aliases=(),
            sim_require_finite=True,
            sim_require_nnan=True,
            nc=nc,
        )
        return tuple(outs)

    devices = jax.devices()[:NCORES]
    assert len(devices) == NCORES
    mesh = Mesh(np.asarray(devices), ("core",))
    in_specs = (PartitionSpec("core"),) * (n_params + n_outs)
    out_specs = (PartitionSpec("core"),) * n_outs
    # No donation: the kernel writes every element of every output, so the
    # zero "out" operands are never read (they exist only to satisfy the NEFF
    # operand list) and can be reused across calls.
    sharded = jax.jit(
        shard_map(_body, mesh=mesh, in_specs=in_specs, out_specs=out_specs,
                  check_rep=False),
        keep_unused=True,
    )

    return {
        "fn": sharded,
        "mesh": mesh,
        "in_names": in_names,
        "out_names": out_names,
        "zero_shapes": zero_shapes,
        "n_params": n_params,
    }


def _get_runner(repeat=1, **buildkw):
    key = (repeat, tuple(sorted(buildkw.items())))
    if key not in _RUNNERS:
        _RUNNERS[key] = _make_runner(_build(repeat, **buildkw))
    return _RUNNERS[key]


F_TILE = 2048                            # must match _build(f_tile=...)
NT = ROWS_PER_CORE // F_TILE


def _concat_inputs(state, A, target):
    # per-core shard, keep the 300 live columns, transpose to feature-major
    # f16, and chunk-interleave x/e/w so the device does one contiguous
    # 12KB-per-partition load per tile: state_dev[100, 3*R] with column
    # blocks [x_i | e_i | w_i] per F_TILE chunk i
    st = np.asarray(state, dtype=np.float32).reshape(NCORES, ROWS_PER_CORE, 4 * DIM)
    live = np.concatenate([st[:, :, :2 * DIM], st[:, :, 3 * DIM:]], axis=2)
    stT = live.transpose(0, 2, 1).astype(np.float16)       # [8, 300, R]
    x = stT[:, 0:DIM].reshape(NCORES, DIM, NT, F_TILE)
    e = stT[:, DIM:2 * DIM].reshape(NCORES, DIM, NT, F_TILE)
    w = stT[:, 2 * DIM:].reshape(NCORES, DIM, NT, F_TILE)
    xew = np.stack([x, e, w], axis=3)                      # [8, 100, NT, 3, F]
    return {
        "state": np.ascontiguousarray(xew).reshape(NCORES * DIM, 3 * ROWS_PER_CORE),
        "A": np.concatenate([A] * NCORES, axis=0),
        "target": np.concatenate([target] * NCORES, axis=0),
    }


def _unpack_out(half):
    # device out: [8*100, 2*R] f16, per-chunk [dx_i | ndx_i] -> [B, 200] f32
    h = np.asarray(half).reshape(NCORES, DIM, NT, 2, F_TILE)
    dxT = h[:, :, :, 0].reshape(NCORES, DIM, ROWS_PER_CORE)
    ndxT = h[:, :, :, 1].reshape(NCORES, DIM, ROWS_PER_CORE)
    out = np.empty((NCORES, ROWS_PER_CORE, 2 * DIM), dtype=np.float32)
    out[:, :, :DIM] = dxT.transpose(0, 2, 1)
    out[:, :, DIM:] = ndxT.transpose(0, 2, 1)
    return out.reshape(BATCH, 2 * DIM)


def run_on_device(state, A, target, repeat=1, n_timed=0, **buildkw):
    """Execute; optionally time n_timed extra calls (device-resident inputs).

    Returns (outT_global [8*200, 16384] f16, times_s list).
    """
    import jax
    from jax.sharding import NamedSharding, PartitionSpec
    import time

    runner = _get_runner(repeat, **buildkw)
    fn = runner["fn"]
    mesh = runner["mesh"]
    shard = NamedSharding(mesh, PartitionSpec("core"))

    cat = _concat_inputs(state, A, target)
    dev_in = [jax.device_put(cat[name], shard) for name in runner["in_names"]]
    dev_z = [
        jax.device_put(np.zeros((NCORES * sh[0], *sh[1:]), dt), shard)
        for (sh, dt) in runner["zero_shapes"]
    ]
    jax.block_until_ready(dev_z)

    outs = fn(*dev_in, *dev_z)
    jax.block_until_ready(outs)
    times = []
    for _ in range(n_timed):
        t0 = time.perf_counter()
        o = fn(*dev_in, *dev_z)
        jax.block_until_ready(o)
        times.append(time.perf_counter() - t0)
    result = np.asarray(outs[0])
    return result, times


def kernel(state, A, target):
    state = np.ascontiguousarray(np.asarray(state, dtype=np.float32))
    A = np.ascontiguousarray(np.asarray(A, dtype=np.float32))
    target = np.ascontiguousarray(np.asarray(target, dtype=np.float32))
    assert state.shape == (BATCH, 4 * DIM)

    half, _ = run_on_device(state, A, target, repeat=1)
    full = np.zeros((BATCH, 4 * DIM), dtype=np.float32)
    full[:, :2 * DIM] = _unpack_out(half)
    return full


# revision 37
# speedup vs baseline: 1.5509x; 1.5509x over previous
"""Trainium2 Bass kernel for nn_CA_event (CA_event.forward batched ODE RHS).

reference:
    x   = state[:, 0:100]
    e_x = state[:, 100:200]
    W_a = state[:, 300:400]          (W_c = state[:, 200:300] unused)
    u   = W_a * (x + e_x - target)
    s   = x^2 / (1 + x^2)
    dx  = -x + s @ A.T + u * s
    out = concat([dx, -dx, 0, 0], axis=-1)      # [B, 400]

Strategy: pure data parallel over 8 NeuronCores (batch 131072 -> 16384
rows/core); A and target replicated.

Layout: the host stages each core's shard FEATURE-MAJOR (transposed) and
f16: state_dev = [300, 16384] = [xT | eT | wT].  This makes the kernel
DMA-roofline-shaped on device:
  * loads/stores are fully contiguous 4KB-per-partition descriptors;
  * the contraction dim of s@A.T lands on partitions, so the matmul runs
    with A.T as a resident stationary operand -- no per-group PE
    transposes, no PSUM->SBUF staging copies;
  * target / sum_k A[j,k] become per-partition scalars, folded into a
    fused scalar_tensor_tensor op and the output writes' bias for free.

Math (rm1 := 1/(1+x^2) - 1 = -s, computed by one fused custom-DVE op:
bitwise-NOT Chebyshev seed + one Newton pass, ~1e-3 rel):
    PSUM_he = I@xT + I@eT            (TensorE identity-matmul accumulation)
    u  = (PSUM_he - tgt) * w         (DVE stt, tgt per-partition scalar)
    t  = rm1 * u  = -u*s             (Pool tensor_mul)
    PSUM = I@xT + I@tT + A.T-matmul(rm1T)        (TensorE, 3 f16 matmuls)
         = xT - (u*s)T - (s@A.T).T = -dxT
  (sum_k A[j,k]*(r[c,k]-1) = -(s@A.T).T[j,c] exactly)
    -dxT -> out[100:200,:]   (ACT copy);   dxT = -PSUM -> out[0:100,:]
  Engine balance per pass: DMA 45.5us (bound) > PE ~38 > ACT ~37 >
  DVE ~33 > Pool ~33 (model).

The device emits only the data-dependent half of the output (dxT | -dxT,
f16, ~5e-4 rel << the 2e-2 gate); the host upcasts/untransposes and
supplies the structurally-zero half (derivatives of W_c / W_a are
identically 0 for any input).
"""

import os
import sys

try:
    import concourse  # noqa: F401  (resolves via the environment's default path)
except ImportError:  # fall back for bare environments
    sys.path.insert(0, "/opt/trn_rl_repo")

import numpy as np

import concourse.bass as bass
import concourse.bacc as bacc
import concourse.mybir as mybir
from concourse import tile
from concourse import masks

DIM = 100
PACK = 3 * DIM                           # xT | eT | wT rows on device
BATCH = 131072
NCORES = 8
ROWS_PER_CORE = BATCH // NCORES          # 16384

F32 = mybir.dt.float32
F16 = mybir.dt.float16

_RUNNERS = {}  # key -> runner dict
_CA_OPS = None


def _register_ca_ops():
    """Register a fused custom-DVE op computing rm1 = 1/(1+x^2) - 1 from x.

    CA_RM1_NR1: in0=x -> r - 1 = -s   (Chebyshev bitwise-NOT seed + 1 NR
    pass, ~1e-3 rel).  Same math/constants as
    dve_ops.RECIPROCAL_APPROX_FAST with the (1 + x^2) denominator
    computation and the final -1 folded in.  Registered at runtime
    (appended to dve_ops.OPS) so no repo files change.
    """
    global _CA_OPS
    if _CA_OPS is not None:
        return _CA_OPS
    from concourse import dve_ops
    from concourse.dve_spec import Spec, Src0, C0, C1, One, Bin, AluOp, sq
    from concourse.dve_uop import DveOpSpec

    d = sq(Src0) + One
    nd = Bin(AluOp.BITWISE_NOT, d, d)
    y0 = nd * C0
    body = y0 * (C1 - d * y0) - One

    def ref(in0, in1, s0, s1, imm2):
        dd = (1.0 + in0.astype(np.float32) * in0).astype(np.float32)
        ndd = (~dd.view(np.int32)).view(np.float32)
        yy0 = (ndd * np.float32(s0)).astype(np.float32)
        return (yy0 * (np.float32(s1) - dd * yy0) - 1.0).astype(np.float32)

    ops = []
    for name, spec in [("CA_RM1_NR1", Spec(body=body, reference=ref))]:
        if name not in dve_ops._SUB_OPCODE_FOR_NAME:
            row = max(dve_ops._SUB_OPCODE_FOR_NAME.values()) + 1
            assert row < 0x20
            dve_ops._SUB_OPCODE_FOR_NAME[name] = row
        shas = {}
        for ver in ("v3", "v4"):
            s = DveOpSpec(
                name=name,
                opcode=dve_ops.get_dve_sub_opcode(name),
                uops=dve_ops.lower(spec, ver=ver),
                rd1_en=dve_ops.has_src1(spec),
            )
            shas[ver] = s.sha(ver)
        op = dve_ops.DveOp(name, spec, subdim=False, uops_sha=shas)
        if not any(o.name == name for o in dve_ops.OPS):
            dve_ops.OPS.append(op)
            dve_ops.CUSTOM_DVE_SPECS[name] = spec
        ops.append(op)
    _CA_OPS = tuple(ops)
    return _CA_OPS


def _build(repeat=1, ablate=(), loop_k=1, f_tile=2048, he_mode="pe",
           u_eng="dve", t_eng="pool", store_ring="split", load_ring="sp",
           body_unroll=8, merged=False):
    """Build the per-core Bacc module.

    he_mode: engine computing he = x + e: 'pool' | 'dve' | 'pe' (PSUM
             identity-matmul accumulation, freeing the elementwise engines)
    u_eng:   engine for u = (he - tgt) * w: 'dve' | 'pool'
             (must be 'dve' when he_mode='pe' -- GpSimd cannot read PSUM)
    t_eng:   engine for t = (r - 1) * u: 'dve' | 'pool'
    body_unroll: passes per For_i iteration when loop_k > 1 -- For_i does an
             all-engine barrier + semaphore reset each iteration (pipeline
             drain); unrolling amortizes it
    ablate: stages to skip for timing experiments only (output wrong):
            'dve', 'pe', 'act', 'load', 'store'
    """
    ablate = set(ablate)
    F = f_tile
    NTILES = ROWS_PER_CORE // F
    CH = 512                              # matmul chunk (one f32 PSUM bank)
    NCH = F // CH
    nc = bacc.Bacc("TRN2", target_bir_lowering=False, debug=False)

    if merged:
        # x/e/w chunk-interleaved on 100 partitions: one 12KB-per-partition
        # load and one 8KB-per-partition store per tile (fewer, bigger DMAs)
        state = nc.declare_dram_parameter("state", [DIM, 3 * ROWS_PER_CORE], F16, isOutput=False)
        out = nc.declare_dram_parameter("out", [DIM, 2 * ROWS_PER_CORE], F16, isOutput=True)
    else:
        state = nc.declare_dram_parameter("state", [PACK, ROWS_PER_CORE], F16, isOutput=False)
        out = nc.declare_dram_parameter("out", [2 * DIM, ROWS_PER_CORE], F16, isOutput=True)
    A = nc.declare_dram_parameter("A", [DIM, DIM], F32, isOutput=False)
    target = nc.declare_dram_parameter("target", [DIM], F32, isOutput=False)

    st_ap = state.ap()
    out_ap = out.ap()

    (op_r,) = _register_ca_ops()

    rings = {"sp": nc.sync, "pool": nc.gpsimd, "act": nc.scalar, "dve": nc.vector}
    ld = rings.get(load_ring, nc.sync)
    sr = rings.get(store_ring)

    with tile.TileContext(nc) as tc:
        with (
            tc.tile_pool(name="consts", bufs=1) as consts,
            tc.tile_pool(name="inp", bufs=3) as inp,
            tc.tile_pool(name="work", bufs=3) as work,
            tc.tile_pool(name="outp", bufs=3) as outp,
            tc.tile_pool(name="psum_mm", bufs=4, space="PSUM") as psum_mm,
        ):
            # ---- one-time constants -------------------------------------
            idf = consts.tile([DIM, DIM], F32)
            masks.make_identity(nc, idf[:])
            id16 = consts.tile([DIM, DIM], F16)
            nc.scalar.copy(id16[:], idf[:])

            a_sb = consts.tile([DIM, DIM], F32)
            nc.sync.dma_start(out=a_sb[:], in_=A.ap())

            # A^T (f16 stationary for the per-chunk matmuls)
            a_ps = psum_mm.tile([DIM, DIM], F32, tag="mm")
            nc.tensor.transpose(a_ps[:], a_sb[:], idf[:])
            at16 = consts.tile([DIM, DIM], F16)
            nc.scalar.copy(at16[:], a_ps[:])

            # target as a per-partition scalar [100, 1]
            tgt = consts.tile([DIM, 1], F32)
            nc.sync.dma_start(out=tgt[:], in_=target.ap()[:, None])

            # ---- main loop ----------------------------------------------
            def emit_pass():
                for i in range(NTILES):
                    sl = slice(i * F, (i + 1) * F)
                    if merged:
                        xew = inp.tile([DIM, 3 * F], F16, tag="xew")
                        if "load" not in ablate:
                            ld.dma_start(out=xew[:],
                                         in_=st_ap[:, i * 3 * F:(i + 1) * 3 * F])
                        xt = xew[:, 0:F]
                        et = xew[:, F:2 * F]
                        wt = xew[:, 2 * F:3 * F]
                    else:
                        xt = inp.tile([DIM, F], F16, tag="x")
                        et = inp.tile([DIM, F], F16, tag="e")
                        wt = inp.tile([DIM, F], F16, tag="w")
                        if "load" not in ablate:
                            if load_ring == "spread":
                                # one load per DMA issue path: SP / ACT /
                                # SWDGE run concurrently
                                nc.sync.dma_start(out=xt[:], in_=st_ap[0:DIM, sl])
                                nc.scalar.dma_start(out=et[:], in_=st_ap[DIM:2 * DIM, sl])
                                nc.gpsimd.dma_start(out=wt[:], in_=st_ap[2 * DIM:3 * DIM, sl])
                            else:
                                ld.dma_start(out=xt[:], in_=st_ap[0:DIM, sl])
                                ld.dma_start(out=et[:], in_=st_ap[DIM:2 * DIM, sl])
                                ld.dma_start(out=wt[:], in_=st_ap[2 * DIM:3 * DIM, sl])

                    if merged:
                        if "act" not in ablate:
                            dn = outp.tile([DIM, 2 * F], F16, tag="dn")
                            dx_sb = dn[:, 0:F]
                            ndx_sb = dn[:, F:2 * F]
                        else:
                            dn = xew[:, 0:2 * F]
                            dx_sb, ndx_sb = xt, et
                    elif "act" not in ablate:
                        dx_sb = outp.tile([DIM, F], F16, tag="dx")
                        ndx_sb = outp.tile([DIM, F], F16, tag="ndx")
                    else:
                        dx_sb, ndx_sb = xt, et   # timing-only: store inputs

                    u = work.tile([DIM, F], F16, tag="u")
                    t = work.tile([DIM, F], F16, tag="t")
                    rm1 = work.tile([DIM, F], F16, tag="rm1")
                    # scalar_tensor_tensor (TensorScalarPtr) is DVE-only on
                    # HW; GpSimd additionally cannot read PSUM.  Pool gets
                    # only plain TensorTensor ops on SBUF.
                    assert u_eng == "dve"
                    t_e = nc.gpsimd if t_eng == "pool" else nc.vector
                    if "dve" not in ablate:
                        # rm1 = 1/(1+x^2) - 1 = -s
                        nc.vector._custom_dve(
                            op_r, out=rm1[:], in0=xt[:],
                            s0=float(np.float32(-0.23549792)),
                            s1=float(np.float32(2.0017324)),
                        )
                        if he_mode == "pe":
                            # he = x + e lives in PSUM via identity matmuls
                            for j in range(NCH):
                                js = slice(j * CH, (j + 1) * CH)
                                ph = psum_mm.tile([DIM, CH], F32, tag="he",
                                                  bufs=2)
                                nc.tensor.matmul(ph[:], id16[:], xt[:, js],
                                                 start=True, stop=False,
                                                 skip_group_check=True)
                                nc.tensor.matmul(ph[:], id16[:], et[:, js],
                                                 start=False, stop=True,
                                                 skip_group_check=True)
                                # u = (he - tgt) * w
                                nc.vector.scalar_tensor_tensor(
                                    u[:, js], ph[:], tgt[:], wt[:, js],
                                    op0=mybir.AluOpType.subtract,
                                    op1=mybir.AluOpType.mult,
                                )
                                # t = rm1 * u = -u*s
                                t_e.tensor_mul(t[:, js], rm1[:, js], u[:, js])
                        else:
                            he = work.tile([DIM, F], F16, tag="he")
                            he_e = nc.gpsimd if he_mode == "pool" else nc.vector
                            he_e.tensor_add(he[:], xt[:], et[:])
                            # u = (he - tgt) * w
                            nc.vector.scalar_tensor_tensor(
                                u[:], he[:], tgt[:], wt[:],
                                op0=mybir.AluOpType.subtract,
                                op1=mybir.AluOpType.mult,
                            )
                            # t = rm1 * u = -u*s
                            t_e.tensor_mul(t[:], rm1[:], u[:])

                    for j in range(NCH):
                        js = slice(j * CH, (j + 1) * CH)
                        mm = psum_mm.tile([DIM, CH], F32, tag="mm")
                        if "pe" not in ablate:
                            nc.tensor.matmul(mm[:], id16[:], xt[:, js],
                                             start=True, stop=False,
                                             skip_group_check=True)
                            nc.tensor.matmul(mm[:], id16[:], t[:, js],
                                             start=False, stop=False,
                                             skip_group_check=True)
                            nc.tensor.matmul(mm[:], at16[:], rm1[:, js],
                                             start=False, stop=True,
                                             skip_group_check=True)
                        if "act" not in ablate:
                            # psum = x - u*s + (A @ rm1T) = -dxT exactly
                            # (sum_k A[j,k](r-1) = -(s@A.T).T)
                            nc.scalar.copy(ndx_sb[:, js], mm[:])
                            nc.scalar.mul(dx_sb[:, js], mm[:], -1.0)

                    if "store" not in ablate:
                        if merged:
                            st_e = nc.scalar if store_ring in ("split", "act") else (
                                nc.gpsimd if store_ring == "pool" else nc.sync)
                            st_e.dma_start(out=out_ap[:, i * 2 * F:(i + 1) * 2 * F],
                                           in_=dn)
                        elif store_ring == "split":
                            nc.scalar.dma_start(out=out_ap[0:DIM, sl], in_=dx_sb[:])
                            nc.sync.dma_start(out=out_ap[DIM:2 * DIM, sl], in_=ndx_sb[:])
                        else:
                            sr.dma_start(out=out_ap[0:DIM, sl], in_=dx_sb[:])
                            sr.dma_start(out=out_ap[DIM:2 * DIM, sl], in_=ndx_sb[:])

            if loop_k > 1:
                stag = bool(int(os.environ.get("CA_STAG", "0")))
                bu = body_unroll
                n_iter = loop_k // bu
                rem = loop_k - n_iter * bu
                if n_iter > 0:
                    with tc.For_i(0, n_iter, 1, staggered_reset=stag):
                        for _ in range(bu):
                            emit_pass()
                for _ in range(rem):
                    emit_pass()
            else:
                for _ in range(repeat):
                    emit_pass()

    nc.compile()
    return nc


def _make_runner(nc):
    """Cached jitted shard_map executor for a prebuilt Bacc module.

    Mirrors bass2jax.run_bass_via_pjrt, but keeps the jitted callable (and
    device-resident inputs) reusable across calls so repeated invocations
    don't re-trace/re-compile.
    """
    import jax
    from jax.experimental.shard_map import shard_map
    from jax.sharding import Mesh, PartitionSpec
    from concourse import bass2jax

    bass2jax.install_neuronx_cc_hook()

    partition_name = nc.partition_id_tensor.name if nc.partition_id_tensor else None
    in_names, out_names, out_avals, zero_shapes = [], [], [], []
    for alloc in nc.m.functions[0].allocations:
        if not isinstance(alloc, mybir.MemoryLocationSet):
            continue
        name = alloc.memorylocations[0].name
        if alloc.kind == "ExternalInput":
            if name != partition_name:
                in_names.append(name)
        elif alloc.kind == "ExternalOutput":
            out_names.append(name)
            shape = tuple(alloc.tensor_shape)
            dtype = mybir.dt.np(alloc.dtype)
            out_avals.append(jax.core.ShapedArray(shape, dtype))
            zero_shapes.append((shape, dtype))
    n_params = len(in_names)
    n_outs = len(out_names)
    bind_in_names = list(in_names) + list(out_names)
    if partition_name is not None:
        bind_in_names.append(partition_name)

    def _body(*args):
        operands = list(args)
        if partition_name is not None:
            operands.append(bass2jax.partition_id_tensor())
        outs = bass2jax._bass_exec_p.bind(
            *operands,
            out_avals=tuple(out_avals),
            in_names=tuple(bind_in_names),
            out_names=tuple(out_names),
            lowering_input_output_aliases=(),
            sim_require_finite=True,
            sim_require_nnan=True,
            nc=nc,
        )
        return tuple(outs)

    devices = jax.devices()[:NCORES]
    assert len(devices) == NCORES
    mesh = Mesh(np.asarray(devices), ("core",))
    in_specs = (PartitionSpec("core"),) * (n_params + n_outs)
    out_specs = (PartitionSpec("core"),) * n_outs
    # No donation: the kernel writes every element of every output, so the
    # zero "out" operands are never read (they exist only to satisfy the NEFF
    # operand list) and can be reused across calls.
    sharded = jax.jit(
        shard_map(_body, mesh=mesh, in_specs=in_specs, out_specs=out_specs,
                  check_rep=False),
        keep_unused=True,
    )

    return {
        "fn": sharded,
        "mesh": mesh,
        "in_names": in_names,
        "out_names": out_names,
        "zero_shapes": zero_shapes,
        "n_params": n_params,
    }


def _get_runner(repeat=1, **buildkw):
    key = (repeat, tuple(sorted(buildkw.items())))
    if key not in _RUNNERS:
        _RUNNERS[key] = _make_runner(_build(repeat, **buildkw))
    return _RUNNERS[key]


F_TILE = 2048                            # must match _build(f_tile=...)
NT = ROWS_PER_CORE // F_TILE
MERGED = False                           # must match _build(merged=...)


def _concat_inputs(state, A, target):
    # per-core shard, keep the 300 live columns, transpose to feature-major,
    # stage f16
    st = np.asarray(state, dtype=np.float32).reshape(NCORES, ROWS_PER_CORE, 4 * DIM)
    live = np.concatenate([st[:, :, :2 * DIM], st[:, :, 3 * DIM:]], axis=2)
    stT = live.transpose(0, 2, 1).astype(np.float16)       # [8, 300, R]
    if MERGED:
        x = stT[:, 0:DIM].reshape(NCORES, DIM, NT, F_TILE)
        e = stT[:, DIM:2 * DIM].reshape(NCORES, DIM, NT, F_TILE)
        w = stT[:, 2 * DIM:].reshape(NCORES, DIM, NT, F_TILE)
        xew = np.stack([x, e, w], axis=3)                  # [8, 100, NT, 3, F]
        st_dev = np.ascontiguousarray(xew).reshape(NCORES * DIM, 3 * ROWS_PER_CORE)
    else:
        st_dev = np.ascontiguousarray(stT).reshape(NCORES * PACK, ROWS_PER_CORE)
    return {
        "state": st_dev,
        "A": np.concatenate([A] * NCORES, axis=0),
        "target": np.concatenate([target] * NCORES, axis=0),
    }


def _unpack_out(half):
    # device out -> [B, 200] f32
    if MERGED:
        h = np.asarray(half).reshape(NCORES, DIM, NT, 2, F_TILE)
        dxT = h[:, :, :, 0].reshape(NCORES, DIM, ROWS_PER_CORE)
        ndxT = h[:, :, :, 1].reshape(NCORES, DIM, ROWS_PER_CORE)
        out = np.empty((NCORES, ROWS_PER_CORE, 2 * DIM), dtype=np.float32)
        out[:, :, :DIM] = dxT.transpose(0, 2, 1)
        out[:, :, DIM:] = ndxT.transpose(0, 2, 1)
        return out.reshape(BATCH, 2 * DIM)
    h = np.asarray(half).reshape(NCORES, 2 * DIM, ROWS_PER_CORE).transpose(0, 2, 1)
    return h.reshape(BATCH, 2 * DIM).astype(np.float32)


def run_on_device(state, A, target, repeat=1, n_timed=0, **buildkw):
    """Execute; optionally time n_timed extra calls (device-resident inputs).

    Returns (outT_global [8*200, 16384] f16, times_s list).
    """
    import jax
    from jax.sharding import NamedSharding, PartitionSpec
    import time

    runner = _get_runner(repeat, **buildkw)
    fn = runner["fn"]
    mesh = runner["mesh"]
    shard = NamedSharding(mesh, PartitionSpec("core"))

    cat = _concat_inputs(state, A, target)
    dev_in = [jax.device_put(cat[name], shard) for name in runner["in_names"]]
    dev_z = [
        jax.device_put(np.zeros((NCORES * sh[0], *sh[1:]), dt), shard)
        for (sh, dt) in runner["zero_shapes"]
    ]
    jax.block_until_ready(dev_z)

    outs = fn(*dev_in, *dev_z)
    jax.block_until_ready(outs)
    times = []
    for _ in range(n_timed):
        t0 = time.perf_counter()
        o = fn(*dev_in, *dev_z)
        jax.block_until_ready(o)
        times.append(time.perf_counter() - t0)
    result = np.asarray(outs[0])
    return result, times


def kernel(state, A, target):
    state = np.ascontiguousarray(np.asarray(state, dtype=np.float32))
    A = np.ascontiguousarray(np.asarray(A, dtype=np.float32))
    target = np.ascontiguousarray(np.asarray(target, dtype=np.float32))
    assert state.shape == (BATCH, 4 * DIM)

    half, _ = run_on_device(state, A, target, repeat=1)
    full = np.zeros((BATCH, 4 * DIM), dtype=np.float32)
    full[:, :2 * DIM] = _unpack_out(half)
    return full


# revision 39
# speedup vs baseline: 1.7831x; 1.1497x over previous
"""Trainium2 Bass kernel for nn_CA_event (CA_event.forward batched ODE RHS).

reference:
    x   = state[:, 0:100]
    e_x = state[:, 100:200]
    W_a = state[:, 300:400]          (W_c = state[:, 200:300] unused)
    u   = W_a * (x + e_x - target)
    s   = x^2 / (1 + x^2)
    dx  = -x + s @ A.T + u * s
    out = concat([dx, -dx, 0, 0], axis=-1)      # [B, 400]

Strategy: pure data parallel over 8 NeuronCores (batch 131072 -> 16384
rows/core); A and target replicated.

Layout: the host stages each core's shard FEATURE-MAJOR (transposed) and
f16: state_dev = [300, 16384] = [xT | eT | wT].  This makes the kernel
DMA-roofline-shaped on device:
  * loads/stores are fully contiguous 4KB-per-partition descriptors;
  * the contraction dim of s@A.T lands on partitions, so the matmul runs
    with A.T as a resident stationary operand -- no per-group PE
    transposes, no PSUM->SBUF staging copies;
  * target / sum_k A[j,k] become per-partition scalars, folded into a
    fused scalar_tensor_tensor op and the output writes' bias for free.

Math (rm1 := 1/(1+x^2) - 1 = -s, computed by one fused custom-DVE op:
bitwise-NOT Chebyshev seed + one Newton pass, ~1e-3 rel):
    PSUM_he = I@xT + I@eT            (TensorE identity-matmul accumulation)
    u  = (PSUM_he - tgt) * w         (DVE stt, tgt per-partition scalar)
    t  = rm1 * u  = -u*s             (Pool tensor_mul)
    PSUM = I@xT + I@tT + A.T-matmul(rm1T)        (TensorE, 3 f16 matmuls)
         = xT - (u*s)T - (s@A.T).T = -dxT
  (sum_k A[j,k]*(r[c,k]-1) = -(s@A.T).T[j,c] exactly)
    -dxT -> out[100:200,:]   (ACT copy);   dxT = -PSUM -> out[0:100,:]
  Engine balance per pass (cost model): DMA 45.5us (bound) > PE ~38 >
  ACT ~37 > DVE ~33 > Pool ~33.  Measured on HW: dma_only 66.9us, full
  ~72us/pass (vs 107.4us baseline) -- HW DMA runs this descriptor
  pattern ~1.45x slower than the model; compute stays hidden.  Fewer,
  larger merged DMAs and spreading loads across SP/ACT/SWDGE rings both
  measured SLOWER (112us / 84us); For_i body_unroll=8 amortizes the
  all-engine loop barrier, 16 was no better.

The device emits only the data-dependent half of the output (dxT | -dxT,
f16, ~5e-4 rel << the 2e-2 gate); the host upcasts/untransposes and
supplies the structurally-zero half (derivatives of W_c / W_a are
identically 0 for any input).
"""

import os
import sys

try:
    import concourse  # noqa: F401  (resolves via the environment's default path)
except ImportError:  # fall back for bare environments
    sys.path.insert(0, "/opt/trn_rl_repo")

import numpy as np

import concourse.bass as bass
import concourse.bacc as bacc
import concourse.mybir as mybir
from concourse import tile
from concourse import masks

DIM = 100
PACK = 3 * DIM                           # xT | eT | wT rows on device
BATCH = 131072
NCORES = 8
ROWS_PER_CORE = BATCH // NCORES          # 16384

F32 = mybir.dt.float32
F16 = mybir.dt.float16

_RUNNERS = {}  # key -> runner dict
_CA_OPS = None


def _register_ca_ops():
    """Register a fused custom-DVE op computing rm1 = 1/(1+x^2) - 1 from x.

    CA_RM1_NR1: in0=x -> r - 1 = -s   (Chebyshev bitwise-NOT seed + 1 NR
    pass, ~1e-3 rel).  Same math/constants as
    dve_ops.RECIPROCAL_APPROX_FAST with the (1 + x^2) denominator
    computation and the final -1 folded in.  Registered at runtime
    (appended to dve_ops.OPS) so no repo files change.
    """
    global _CA_OPS
    if _CA_OPS is not None:
        return _CA_OPS
    from concourse import dve_ops
    from concourse.dve_spec import Spec, Src0, C0, C1, One, Bin, AluOp, sq
    from concourse.dve_uop import DveOpSpec

    d = sq(Src0) + One
    nd = Bin(AluOp.BITWISE_NOT, d, d)
    y0 = nd * C0
    body = y0 * (C1 - d * y0) - One

    def ref(in0, in1, s0, s1, imm2):
        dd = (1.0 + in0.astype(np.float32) * in0).astype(np.float32)
        ndd = (~dd.view(np.int32)).view(np.float32)
        yy0 = (ndd * np.float32(s0)).astype(np.float32)
        return (yy0 * (np.float32(s1) - dd * yy0) - 1.0).astype(np.float32)

    ops = []
    for name, spec in [("CA_RM1_NR1", Spec(body=body, reference=ref))]:
        if name not in dve_ops._SUB_OPCODE_FOR_NAME:
            row = max(dve_ops._SUB_OPCODE_FOR_NAME.values()) + 1
            assert row < 0x20
            dve_ops._SUB_OPCODE_FOR_NAME[name] = row
        shas = {}
        for ver in ("v3", "v4"):
            s = DveOpSpec(
                name=name,
                opcode=dve_ops.get_dve_sub_opcode(name),
                uops=dve_ops.lower(spec, ver=ver),
                rd1_en=dve_ops.has_src1(spec),
            )
            shas[ver] = s.sha(ver)
        op = dve_ops.DveOp(name, spec, subdim=False, uops_sha=shas)
        if not any(o.name == name for o in dve_ops.OPS):
            dve_ops.OPS.append(op)
            dve_ops.CUSTOM_DVE_SPECS[name] = spec
        ops.append(op)
    _CA_OPS = tuple(ops)
    return _CA_OPS


def _build(repeat=1, ablate=(), loop_k=1, f_tile=1024, he_mode="pe",
           u_eng="dve", t_eng="pool", store_ring="pool", load_ring="sp",
           body_unroll=8, merged=False):
    """Build the per-core Bacc module.

    he_mode: engine computing he = x + e: 'pool' | 'dve' | 'pe' (PSUM
             identity-matmul accumulation, freeing the elementwise engines)
    u_eng:   engine for u = (he - tgt) * w: 'dve' | 'pool'
             (must be 'dve' when he_mode='pe' -- GpSimd cannot read PSUM)
    t_eng:   engine for t = (r - 1) * u: 'dve' | 'pool'
    body_unroll: passes per For_i iteration when loop_k > 1 -- For_i does an
             all-engine barrier + semaphore reset each iteration (pipeline
             drain); unrolling amortizes it
    ablate: stages to skip for timing experiments only (output wrong):
            'dve', 'pe', 'act', 'load', 'store'
    """
    ablate = set(ablate)
    F = f_tile
    NTILES = ROWS_PER_CORE // F
    CH = 512                              # matmul chunk (one f32 PSUM bank)
    NCH = F // CH
    nc = bacc.Bacc("TRN2", target_bir_lowering=False, debug=False)

    if merged:
        # x/e/w chunk-interleaved on 100 partitions: one 12KB-per-partition
        # load and one 8KB-per-partition store per tile (fewer, bigger DMAs)
        state = nc.declare_dram_parameter("state", [DIM, 3 * ROWS_PER_CORE], F16, isOutput=False)
        out = nc.declare_dram_parameter("out", [DIM, 2 * ROWS_PER_CORE], F16, isOutput=True)
    else:
        state = nc.declare_dram_parameter("state", [PACK, ROWS_PER_CORE], F16, isOutput=False)
        out = nc.declare_dram_parameter("out", [2 * DIM, ROWS_PER_CORE], F16, isOutput=True)
    A = nc.declare_dram_parameter("A", [DIM, DIM], F32, isOutput=False)
    target = nc.declare_dram_parameter("target", [DIM], F32, isOutput=False)

    st_ap = state.ap()
    out_ap = out.ap()

    (op_r,) = _register_ca_ops()

    rings = {"sp": nc.sync, "pool": nc.gpsimd, "act": nc.scalar, "dve": nc.vector}
    ld = rings.get(load_ring, nc.sync)
    sr = rings.get(store_ring)

    with tile.TileContext(nc) as tc:
        with (
            tc.tile_pool(name="consts", bufs=1) as consts,
            tc.tile_pool(name="inp", bufs=3) as inp,
            tc.tile_pool(name="work", bufs=3) as work,
            tc.tile_pool(name="outp", bufs=3) as outp,
            tc.tile_pool(name="psum_mm", bufs=4, space="PSUM") as psum_mm,
        ):
            # ---- one-time constants -------------------------------------
            idf = consts.tile([DIM, DIM], F32)
            masks.make_identity(nc, idf[:])
            id16 = consts.tile([DIM, DIM], F16)
            nc.scalar.copy(id16[:], idf[:])

            a_sb = consts.tile([DIM, DIM], F32)
            nc.sync.dma_start(out=a_sb[:], in_=A.ap())

            # A^T (f16 stationary for the per-chunk matmuls)
            a_ps = psum_mm.tile([DIM, DIM], F32, tag="mm")
            nc.tensor.transpose(a_ps[:], a_sb[:], idf[:])
            at16 = consts.tile([DIM, DIM], F16)
            nc.scalar.copy(at16[:], a_ps[:])

            # target as a per-partition scalar [100, 1]
            tgt = consts.tile([DIM, 1], F32)
            nc.sync.dma_start(out=tgt[:], in_=target.ap()[:, None])

            # ---- main loop ----------------------------------------------
            def emit_pass():
                for i in range(NTILES):
                    sl = slice(i * F, (i + 1) * F)
                    if merged:
                        xew = inp.tile([DIM, 3 * F], F16, tag="xew")
                        if "load" not in ablate:
                            ld.dma_start(out=xew[:],
                                         in_=st_ap[:, i * 3 * F:(i + 1) * 3 * F])
                        xt = xew[:, 0:F]
                        et = xew[:, F:2 * F]
                        wt = xew[:, 2 * F:3 * F]
                    else:
                        xt = inp.tile([DIM, F], F16, tag="x")
                        et = inp.tile([DIM, F], F16, tag="e")
                        wt = inp.tile([DIM, F], F16, tag="w")
                        if "load" not in ablate:
                            if load_ring == "spread":
                                # one load per DMA issue path: SP / ACT /
                                # SWDGE run concurrently
                                nc.sync.dma_start(out=xt[:], in_=st_ap[0:DIM, sl])
                                nc.scalar.dma_start(out=et[:], in_=st_ap[DIM:2 * DIM, sl])
                                nc.gpsimd.dma_start(out=wt[:], in_=st_ap[2 * DIM:3 * DIM, sl])
                            else:
                                ld.dma_start(out=xt[:], in_=st_ap[0:DIM, sl])
                                ld.dma_start(out=et[:], in_=st_ap[DIM:2 * DIM, sl])
                                ld.dma_start(out=wt[:], in_=st_ap[2 * DIM:3 * DIM, sl])

                    if merged:
                        if "act" not in ablate:
                            dn = outp.tile([DIM, 2 * F], F16, tag="dn")
                            dx_sb = dn[:, 0:F]
                            ndx_sb = dn[:, F:2 * F]
                        else:
                            dn = xew[:, 0:2 * F]
                            dx_sb, ndx_sb = xt, et
                    elif "act" not in ablate:
                        dx_sb = outp.tile([DIM, F], F16, tag="dx")
                        ndx_sb = outp.tile([DIM, F], F16, tag="ndx")
                    else:
                        dx_sb, ndx_sb = xt, et   # timing-only: store inputs

                    u = work.tile([DIM, F], F16, tag="u")
                    t = work.tile([DIM, F], F16, tag="t")
                    rm1 = work.tile([DIM, F], F16, tag="rm1")
                    # scalar_tensor_tensor (TensorScalarPtr) is DVE-only on
                    # HW; GpSimd additionally cannot read PSUM.  Pool gets
                    # only plain TensorTensor ops on SBUF.
                    assert u_eng == "dve"
                    t_e = nc.gpsimd if t_eng == "pool" else nc.vector
                    if "dve" not in ablate:
                        # rm1 = 1/(1+x^2) - 1 = -s
                        nc.vector._custom_dve(
                            op_r, out=rm1[:], in0=xt[:],
                            s0=float(np.float32(-0.23549792)),
                            s1=float(np.float32(2.0017324)),
                        )
                        if he_mode == "pe":
                            # he = x + e lives in PSUM via identity matmuls
                            for j in range(NCH):
                                js = slice(j * CH, (j + 1) * CH)
                                ph = psum_mm.tile([DIM, CH], F32, tag="he",
                                                  bufs=2)
                                nc.tensor.matmul(ph[:], id16[:], xt[:, js],
                                                 start=True, stop=False,
                                                 skip_group_check=True)
                                nc.tensor.matmul(ph[:], id16[:], et[:, js],
                                                 start=False, stop=True,
                                                 skip_group_check=True)
                                # u = (he - tgt) * w
                                nc.vector.scalar_tensor_tensor(
                                    u[:, js], ph[:], tgt[:], wt[:, js],
                                    op0=mybir.AluOpType.subtract,
                                    op1=mybir.AluOpType.mult,
                                )
                                # t = rm1 * u = -u*s
                                t_e.tensor_mul(t[:, js], rm1[:, js], u[:, js])
                        else:
                            he = work.tile([DIM, F], F16, tag="he")
                            he_e = nc.gpsimd if he_mode == "pool" else nc.vector
                            he_e.tensor_add(he[:], xt[:], et[:])
                            # u = (he - tgt) * w
                            nc.vector.scalar_tensor_tensor(
                                u[:], he[:], tgt[:], wt[:],
                                op0=mybir.AluOpType.subtract,
                                op1=mybir.AluOpType.mult,
                            )
                            # t = rm1 * u = -u*s
                            t_e.tensor_mul(t[:], rm1[:], u[:])

                    for j in range(NCH):
                        js = slice(j * CH, (j + 1) * CH)
                        mm = psum_mm.tile([DIM, CH], F32, tag="mm")
                        if "pe" not in ablate:
                            nc.tensor.matmul(mm[:], id16[:], xt[:, js],
                                             start=True, stop=False,
                                             skip_group_check=True)
                            nc.tensor.matmul(mm[:], id16[:], t[:, js],
                                             start=False, stop=False,
                                             skip_group_check=True)
                            nc.tensor.matmul(mm[:], at16[:], rm1[:, js],
                                             start=False, stop=True,
                                             skip_group_check=True)
                        if "act" not in ablate:
                            # psum = x - u*s + (A @ rm1T) = -dxT exactly
                            # (sum_k A[j,k](r-1) = -(s@A.T).T)
                            nc.scalar.copy(ndx_sb[:, js], mm[:])
                            nc.scalar.mul(dx_sb[:, js], mm[:], -1.0)

                    if "store" not in ablate:
                        if merged:
                            st_e = nc.scalar if store_ring in ("split", "act") else (
                                nc.gpsimd if store_ring == "pool" else nc.sync)
                            st_e.dma_start(out=out_ap[:, i * 2 * F:(i + 1) * 2 * F],
                                           in_=dn)
                        elif store_ring == "split":
                            nc.scalar.dma_start(out=out_ap[0:DIM, sl], in_=dx_sb[:])
                            nc.sync.dma_start(out=out_ap[DIM:2 * DIM, sl], in_=ndx_sb[:])
                        else:
                            sr.dma_start(out=out_ap[0:DIM, sl], in_=dx_sb[:])
                            sr.dma_start(out=out_ap[DIM:2 * DIM, sl], in_=ndx_sb[:])

            if loop_k > 1:
                stag = bool(int(os.environ.get("CA_STAG", "0")))
                bu = body_unroll
                n_iter = loop_k // bu
                rem = loop_k - n_iter * bu
                if n_iter > 0:
                    with tc.For_i(0, n_iter, 1, staggered_reset=stag):
                        for _ in range(bu):
                            emit_pass()
                for _ in range(rem):
                    emit_pass()
            else:
                for _ in range(repeat):
                    emit_pass()

    nc.compile()
    return nc


def _make_runner(nc):
    """Cached jitted shard_map executor for a prebuilt Bacc module.

    Mirrors bass2jax.run_bass_via_pjrt, but keeps the jitted callable (and
    device-resident inputs) reusable across calls so repeated invocations
    don't re-trace/re-compile.
    """
    import jax
    from jax.experimental.shard_map import shard_map
    from jax.sharding import Mesh, PartitionSpec
    from concourse import bass2jax

    bass2jax.install_neuronx_cc_hook()

    partition_name = nc.partition_id_tensor.name if nc.partition_id_tensor else None
    in_names, out_names, out_avals, zero_shapes = [], [], [], []
    for alloc in nc.m.functions[0].allocations:
        if not isinstance(alloc, mybir.MemoryLocationSet):
            continue
        name = alloc.memorylocations[0].name
        if alloc.kind == "ExternalInput":
            if name != partition_name:
                in_names.append(name)
        elif alloc.kind == "ExternalOutput":
            out_names.append(name)
            shape = tuple(alloc.tensor_shape)
            dtype = mybir.dt.np(alloc.dtype)
            out_avals.append(jax.core.ShapedArray(shape, dtype))
            zero_shapes.append((shape, dtype))
    n_params = len(in_names)
    n_outs = len(out_names)
    bind_in_names = list(in_names) + list(out_names)
    if partition_name is not None:
        bind_in_names.append(partition_name)

    def _body(*args):
        operands = list(args)
        if partition_name is not None:
            operands.append(bass2jax.partition_id_tensor())
        outs = bass2jax._bass_exec_p.bind(
            *operands,
            out_avals=tuple(out_avals),
            in_names=tuple(bind_in_names),
            out_names=tuple(out_names),
            lowering_input_output_aliases=(),
            sim_require_finite=True,
            sim_require_nnan=True,
            nc=nc,
        )
        return tuple(outs)

    devices = jax.devices()[:NCORES]
    assert len(devices) == NCORES
    mesh = Mesh(np.asarray(devices), ("core",))
    in_specs = (PartitionSpec("core"),) * (n_params + n_outs)
    out_specs = (PartitionSpec("core"),) * n_outs
    # No donation: the kernel writes every element of every output, so the
    # zero "out" operands are never read (they exist only to satisfy the NEFF
    # operand list) and can be reused across calls.
    sharded = jax.jit(
        shard_map(_body, mesh=mesh, in_specs=in_specs, out_specs=out_specs,
                  check_rep=False),
        keep_unused=True,
    )

    return {
        "fn": sharded,
        "mesh": mesh,
        "in_names": in_names,
        "out_names": out_names,
        "zero_shapes": zero_shapes,
        "n_params": n_params,
    }


def _get_runner(repeat=1, **buildkw):
    key = (repeat, tuple(sorted(buildkw.items())))
    if key not in _RUNNERS:
        _RUNNERS[key] = _make_runner(_build(repeat, **buildkw))
    return _RUNNERS[key]


F_TILE = 2048                            # must match _build(f_tile=...)
NT = ROWS_PER_CORE // F_TILE
MERGED = False                           # must match _build(merged=...)


def _concat_inputs(state, A, target):
    # per-core shard, keep the 300 live columns, transpose to feature-major,
    # stage f16
    st = np.asarray(state, dtype=np.float32).reshape(NCORES, ROWS_PER_CORE, 4 * DIM)
    live = np.concatenate([st[:, :, :2 * DIM], st[:, :, 3 * DIM:]], axis=2)
    stT = live.transpose(0, 2, 1).astype(np.float16)       # [8, 300, R]
    if MERGED:
        x = stT[:, 0:DIM].reshape(NCORES, DIM, NT, F_TILE)
        e = stT[:, DIM:2 * DIM].reshape(NCORES, DIM, NT, F_TILE)
        w = stT[:, 2 * DIM:].reshape(NCORES, DIM, NT, F_TILE)
        xew = np.stack([x, e, w], axis=3)                  # [8, 100, NT, 3, F]
        st_dev = np.ascontiguousarray(xew).reshape(NCORES * DIM, 3 * ROWS_PER_CORE)
    else:
        st_dev = np.ascontiguousarray(stT).reshape(NCORES * PACK, ROWS_PER_CORE)
    return {
        "state": st_dev,
        "A": np.concatenate([A] * NCORES, axis=0),
        "target": np.concatenate([target] * NCORES, axis=0),
    }


def _unpack_out(half):
    # device out -> [B, 200] f32
    if MERGED:
        h = np.asarray(half).reshape(NCORES, DIM, NT, 2, F_TILE)
        dxT = h[:, :, :, 0].reshape(NCORES, DIM, ROWS_PER_CORE)
        ndxT = h[:, :, :, 1].reshape(NCORES, DIM, ROWS_PER_CORE)
        out = np.empty((NCORES, ROWS_PER_CORE, 2 * DIM), dtype=np.float32)
        out[:, :, :DIM] = dxT.transpose(0, 2, 1)
        out[:, :, DIM:] = ndxT.transpose(0, 2, 1)
        return out.reshape(BATCH, 2 * DIM)
    h = np.asarray(half).reshape(NCORES, 2 * DIM, ROWS_PER_CORE).transpose(0, 2, 1)
    return h.reshape(BATCH, 2 * DIM).astype(np.float32)


def run_on_device(state, A, target, repeat=1, n_timed=0, **buildkw):
    """Execute; optionally time n_timed extra calls (device-resident inputs).

    Returns (outT_global [8*200, 16384] f16, times_s list).
    """
    import jax
    from jax.sharding import NamedSharding, PartitionSpec
    import time

    runner = _get_runner(repeat, **buildkw)
    fn = runner["fn"]
    mesh = runner["mesh"]
    shard = NamedSharding(mesh, PartitionSpec("core"))

    cat = _concat_inputs(state, A, target)
    dev_in = [jax.device_put(cat[name], shard) for name in runner["in_names"]]
    dev_z = [
        jax.device_put(np.zeros((NCORES * sh[0], *sh[1:]), dt), shard)
        for (sh, dt) in runner["zero_shapes"]
    ]
    jax.block_until_ready(dev_z)

    outs = fn(*dev_in, *dev_z)
    jax.block_until_ready(outs)
    times = []
    for _ in range(n_timed):
        t0 = time.perf_counter()
        o = fn(*dev_in, *dev_z)
        jax.block_until_ready(o)
        times.append(time.perf_counter() - t0)
    result = np.asarray(outs[0])
    return result, times


def kernel(state, A, target):
    state = np.ascontiguousarray(np.asarray(state, dtype=np.float32))
    A = np.ascontiguousarray(np.asarray(A, dtype=np.float32))
    target = np.ascontiguousarray(np.asarray(target, dtype=np.float32))
    assert state.shape == (BATCH, 4 * DIM)

    half, _ = run_on_device(state, A, target, repeat=1)
    full = np.zeros((BATCH, 4 * DIM), dtype=np.float32)
    full[:, :2 * DIM] = _unpack_out(half)
    return full
